# revision 2
# baseline (speedup 1.0000x reference)
"""CBOW (one-hot embedding lookup + mean + output matmul + softmax) on 8
Trainium2 NeuronCores, vocab-sharded end to end.

Full problem: batch [1024, 10, 32000] f32 one-hot, emb [32000, 128] f32,
w_out [128, 32000] f32 -> softmax(mean_c(batch @ emb) @ w_out) [1024, 32000].

Sharding: core i owns vocab columns [i*4000, (i+1)*4000). It receives
  batch_s [1024, 10, 4000] f32  (full batch, its vocab slice)
  emb_s   [4096, 128]      f32  (its emb rows, zero-padded 4000->4096)
  w_out_s [128, 4000]      f32  (its output-projection columns)
and produces out_s [1024, 4000] bf16 (its softmax columns; host concatenates
along vocab and upcasts to f32).

Per core, batch rows are processed in 8 blocks of 128, pipelined:
  stage 1 (per block): stream one-hot v-chunks through a casting DMA (f32
          DRAM -> bf16 SBUF). Per 128-wide v-tile, sum the 10 context
          planes on the PE as REGULAR bf16 matmuls (lhsT=oh_c,
          rhs=identity -> accumulates oh_c.T in fp32 PSUM), giving
          sT[v, b]; then avgT_bb[d, b] += emb_tile.T @ sT accumulates over
          the core's 32 v-tiles (31x128 + 1x32).
  avg all-reduce (per block): partial avgT_bb (scaled by 1/C) bounces
          SBUF -> DRAM -> AllReduce over all 8 cores -> SBUF. 64 KB each,
          overlapped with the next block's stage-1 streaming.
  stage 2 (per block): logits chunk [128, 512] = avgT_g.T @ w_out chunk on
          the PE; exp reads PSUM on the scalar engine (logits bounded ~|22|
          here, so fp32 exp without max subtraction is safe) and lands in
          bf16 SBUF; DVE accumulates per-chunk sums into a local partial
          softmax denominator, which takes a tiny [128,1] AllReduce; then
          scale by 1/sum and DMA out as bf16.
"""

from contextlib import ExitStack

import numpy as np

import concourse.bass as bass
import concourse.tile as tile
from concourse import bacc, masks, mybir
from concourse._compat import with_exitstack

F32 = mybir.dt.float32
BF16 = mybir.dt.bfloat16
AX = mybir.AxisListType
AF = mybir.ActivationFunctionType

B_FULL, C, V, D = 1024, 10, 32000, 128
N_CORES = 8
VS = V // N_CORES          # 4000 vocab columns per core
VS_PAD = 4096              # emb rows padded to a multiple of 128
N_TILES = VS_PAD // 128    # 32 v-tiles (last is 32 valid rows)
BB = 128                   # batch rows per block
N_BB = B_FULL // BB        # 8 blocks


@with_exitstack
def _cbow_kernel(ctx: ExitStack, tc, out, batch, emb, w_out, VC=1024, NC2=512):
    nc = tc.nc
    Bs, Cs, Vs = batch.shape
    assert Bs == B_FULL and Cs == C and Vs == VS
    rg = [list(range(N_CORES))]

    const_pool = ctx.enter_context(tc.tile_pool(name="const", bufs=1))
    ident = const_pool.tile([128, 128], BF16)
    masks.make_identity(nc, ident[:])

    # replicated weights, loaded once up front
    eb_pool = ctx.enter_context(tc.tile_pool(name="eb", bufs=1))
    eb = eb_pool.tile([128, N_TILES, 128], F32)
    nc.sync.dma_start(eb[:], emb.rearrange("(n p) d -> p n d", p=128))
    wo_pool = ctx.enter_context(tc.tile_pool(name="wo", bufs=1))
    wo = wo_pool.tile([128, VS], F32)
    nc.sync.dma_start(wo[:], w_out)

    oh_pool = ctx.enter_context(tc.tile_pool(name="oh", bufs=3))
    sT_pool = ctx.enter_context(tc.tile_pool(name="sT", bufs=4))
    sTps_pool = ctx.enter_context(tc.tile_pool(name="sTps", bufs=3, space="PSUM"))
    acc_pool = ctx.enter_context(tc.tile_pool(name="acc", bufs=2, space="PSUM"))
    avg_pool = ctx.enter_context(tc.tile_pool(name="avg", bufs=2))
    avgg_pool = ctx.enter_context(tc.tile_pool(name="avgg", bufs=3))
    lg_pool = ctx.enter_context(tc.tile_pool(name="lg", bufs=3))
    lgps_pool = ctx.enter_context(tc.tile_pool(name="lgps", bufs=3, space="PSUM"))
    stat_pool = ctx.enter_context(tc.tile_pool(name="stat", bufs=3))
    dram = ctx.enter_context(tc.tile_pool(name="dram", bufs=8, space="DRAM"))
    dram2 = ctx.enter_context(tc.tile_pool(name="dram2", bufs=8, space="DRAM"))

    n_vc = (Vs + VC - 1) // VC
    n_nc = (Vs + NC2 - 1) // NC2

    for bb in range(N_BB):
        b0 = bb * BB
        # ---- stage 1: sum one-hot context planes, project through emb ----
        avgT_ps = acc_pool.tile([128, BB], F32, tag="acc")
        g = 0
        for j in range(n_vc):
            v0 = j * VC
            vc = min(VC, Vs - v0)
            oh = oh_pool.tile([128, Cs, VC], BF16, tag="oh")
            nc.gpsimd.dma_start(
                oh[:, :, :vc], batch[b0 : b0 + BB, :, v0 : v0 + vc]
            )
            nt = (vc + 127) // 128
            for t in range(nt):
                toff = t * 128
                tw = min(128, vc - toff)
                sT_ps = sTps_pool.tile([128, BB], F32, tag="sTps")
                for c in range(Cs):
                    nc.tensor.matmul(
                        sT_ps[:tw],
                        lhsT=oh[:, c, toff : toff + tw],
                        rhs=ident[:],
                        start=(c == 0),
                        stop=(c == Cs - 1),
                    )
                sT = sT_pool.tile([128, BB], F32, tag="sT")
                nc.vector.tensor_copy(sT[:tw], sT_ps[:tw])
                nc.tensor.matmul(
                    avgT_ps[:],
                    lhsT=eb[:tw, g, :],
                    rhs=sT[:tw],
                    start=(g == 0),
                    stop=(g == N_TILES - 1),
                )
                g += 1
        assert g == N_TILES

        # ---- all-reduce the partial average embedding for this block ----
        avgT_sb = avg_pool.tile([128, BB], F32, tag="avg")
        nc.vector.tensor_scalar_mul(avgT_sb[:], avgT_ps[:], 1.0 / Cs)
        cc_in = dram.tile([128, BB], F32, tag="cc_in")
        cc_out = dram.tile([128, BB], F32, tag="cc_out", addr_space="Shared")
        nc.sync.dma_start(cc_in[:], avgT_sb[:])
        nc.gpsimd.collective_compute(
            "AllReduce",
            mybir.AluOpType.add,
            replica_groups=rg,
            ins=[cc_in.opt()],
            outs=[cc_out.opt()],
        )
        avgT_g = avgg_pool.tile([128, BB], F32, tag="avgg")
        nc.sync.dma_start(avgT_g[:], cc_out[:])

        # ---- stage 2: logits chunk, exp, local denominator ----
        lg = lg_pool.tile([128, VS], BF16, tag="lg")
        sums = stat_pool.tile([128, n_nc], F32, tag="sums")
        for k in range(n_nc):
            n0 = k * NC2
            nw = min(NC2, Vs - n0)
            lg_ps = lgps_pool.tile([128, NC2], F32, tag="lgps")
            nc.tensor.matmul(
                lg_ps[:, :nw],
                lhsT=avgT_g[:],
                rhs=wo[:, n0 : n0 + nw],
                start=True,
                stop=True,
            )
            nc.scalar.activation(
                lg[:, n0 : n0 + nw], lg_ps[:, :nw], AF.Exp, scale=1.0
            )
            nc.vector.tensor_reduce(
                sums[:, k : k + 1],
                lg[:, n0 : n0 + nw],
                axis=AX.X,
                op=mybir.AluOpType.add,
            )
        den = stat_pool.tile([128, 1], F32, tag="den")
        nc.vector.tensor_reduce(
            den[:], sums[:, :n_nc], axis=AX.X, op=mybir.AluOpType.add
        )

        # ---- tiny all-reduce of the softmax denominator ----
        cc2_in = dram2.tile([128, 1], F32, tag="cc2_in")
        cc2_out = dram2.tile([128, 1], F32, tag="cc2_out", addr_space="Shared")
        nc.sync.dma_start(cc2_in[:], den[:])
        nc.gpsimd.collective_compute(
            "AllReduce",
            mybir.AluOpType.add,
            replica_groups=rg,
            ins=[cc2_in.opt()],
            outs=[cc2_out.opt()],
        )
        den_g = stat_pool.tile([128, 1], F32, tag="deng")
        nc.sync.dma_start(den_g[:], cc2_out[:])
        r = stat_pool.tile([128, 1], F32, tag="recip")
        nc.vector.reciprocal(r[:], den_g[:])

        # ---- scale and write out (bf16) ----
        for k in range(n_nc):
            n0 = k * NC2
            nw = min(NC2, Vs - n0)
            nc.vector.tensor_scalar_mul(
                lg[:, n0 : n0 + nw], lg[:, n0 : n0 + nw], r[:]
            )
        nc.sync.dma_start(out[b0 : b0 + BB, :], lg[:])


def build(VC=1024, NC2=512, num_devices=N_CORES):
    nc = bacc.Bacc(
        "TRN2",
        target_bir_lowering=False,
        debug=False,
        num_devices=num_devices,
        num_swdge_queues=4,
    )
    batch = nc.dram_tensor(
        "batch", [B_FULL, C, VS], F32, kind="ExternalInput"
    ).ap()
    emb = nc.dram_tensor("emb", [VS_PAD, D], F32, kind="ExternalInput").ap()
    w_out = nc.dram_tensor("w_out", [D, VS], F32, kind="ExternalInput").ap()
    out = nc.dram_tensor("out", [B_FULL, VS], BF16, kind="ExternalOutput").ap()
    with tile.TileContext(nc) as tc:
        _cbow_kernel(tc, out, batch, emb, w_out, VC=VC, NC2=NC2)
    nc.compile()
    return nc


_NC = None


def _build_cached():
    global _NC
    if _NC is None:
        _NC = build()
    return _NC


def _run(batch, emb, w_out, trace=False, **kwargs):
    from concourse.bass_utils import run_bass_kernel_spmd

    nc = _build_cached()
    batch = np.ascontiguousarray(np.asarray(batch, dtype=np.float32))
    emb = np.asarray(emb, dtype=np.float32)
    w_out = np.asarray(w_out, dtype=np.float32)
    in_maps = []
    for i in range(N_CORES):
        v0 = i * VS
        emb_pad = np.zeros((VS_PAD, D), dtype=np.float32)
        emb_pad[:VS] = emb[v0 : v0 + VS]
        in_maps.append(
            {
                "batch": np.ascontiguousarray(batch[:, :, v0 : v0 + VS]),
                "emb": emb_pad,
                "w_out": np.ascontiguousarray(w_out[:, v0 : v0 + VS]),
            }
        )
    res = run_bass_kernel_spmd(
        nc, in_maps, core_ids=list(range(N_CORES)), trace=trace, **kwargs
    )
    out = np.concatenate(
        [r["out"].astype(np.float32) for r in res.results], axis=1
    )
    return out, res


def kernel(batch, emb, w_out):
    out, _ = _run(batch, emb, w_out, trace=False)
    return out


# revision 3
# speedup vs baseline: 1.0079x; 1.0079x over previous
"""CBOW (one-hot embedding lookup + mean + output matmul + softmax) on 8
Trainium2 NeuronCores, vocab-sharded end to end.

Full problem: batch [1024, 10, 32000] f32 one-hot, emb [32000, 128] f32,
w_out [128, 32000] f32 -> softmax(mean_c(batch @ emb) @ w_out) [1024, 32000].

Sharding: core i owns vocab columns [i*4000, (i+1)*4000). It receives
  batch_s [1024, 10, 4000] f32  (full batch, its vocab slice)
  emb_s   [4096, 128]      f32  (its emb rows, zero-padded 4000->4096)
  w_out_s [128, 4000]      f32  (its output-projection columns)
and produces out_s [1024, 4000] bf16 (its softmax columns; host concatenates
along vocab and upcasts to f32).

Batch rows run in 8 blocks of 128, software-pipelined so block bb-1's
output stage hides entirely under block bb's one-hot streaming:

  stage 1 (block bb): stream one-hot v-chunks through a casting DMA (f32
      DRAM -> bf16 SBUF). Per 128-wide v-tile, sum the 10 context planes on
      the PE as REGULAR bf16 matmuls (lhsT=oh_c, rhs=identity -> accumulates
      oh_c.T in fp32 PSUM) giving sT[v, b]; then sumT_bb[d, b] +=
      emb_tile.T @ sT over the core's 32 v-tiles. The only DVE work is
      PSUM->SBUF copies, so DVE never waits on collectives.
  avg all-reduce (block bb, end of window): context-SUM (not mean; the 1/C
      folds into the exp scale) bounces SBUF -> DRAM -> 64 KB AllReduce ->
      SBUF, overlapping block bb+1's streaming.
  stage 2A (block bb-1, after chunk 1 of bb): logits chunks [128, 512] =
      avgT_g.T @ w_out chunk on the PE; scalar-engine exp(0.1*x) reads PSUM
      and lands bf16 in SBUF with the row-sum fused via accum_out; a tiny
      Copy+accum_out folds the 8 chunk sums; the [128,1] denominator
      all-reduce is triggered. All on engines with slack, after their
      dependencies are already met -> no head-of-line stalls.
  stage 2B (block bb-1, after chunk 3 of bb): reciprocal of the global
      denominator (DVE, long since available), per-chunk scale on the
      scalar engine (Copy activation with per-partition scale), bf16 DMA
      out.

Engine roles: gpsimd = one-hot casting DMAs + collective triggers (in the
staggered order avg0, den0, avg1, den1, ... with den(bb) issued a half
window after avg(bb) so the in-order collective queue never blocks);
sync = weight loads, collective bounces, output writes (no streaming, may
block freely); scalar = exp/sums/scales; vector = PSUM copies + recip.
"""

from contextlib import ExitStack

import numpy as np

import concourse.bass as bass
import concourse.tile as tile
from concourse import bacc, masks, mybir
from concourse._compat import with_exitstack

F32 = mybir.dt.float32
BF16 = mybir.dt.bfloat16
AX = mybir.AxisListType
AF = mybir.ActivationFunctionType

B_FULL, C, V, D = 1024, 10, 32000, 128
N_CORES = 8
VS = V // N_CORES          # 4000 vocab columns per core
VS_PAD = 4096              # emb rows padded to a multiple of 128
N_TILES = VS_PAD // 128    # 32 v-tiles (last is 32 valid rows)
BB = 128                   # batch rows per block
N_BB = B_FULL // BB        # 8 blocks
VC = 1024                  # one-hot v-chunk (chunks: 1024,1024,1024,928)
NC2 = 512                  # stage-2 logits chunk


@with_exitstack
def _cbow_kernel(ctx: ExitStack, tc, out, batch, emb, w_out):
    nc = tc.nc
    Bs, Cs, Vs = batch.shape
    assert Bs == B_FULL and Cs == C and Vs == VS
    rg = [list(range(N_CORES))]
    n_vc = (Vs + VC - 1) // VC
    n_nc = (Vs + NC2 - 1) // NC2

    const_pool = ctx.enter_context(tc.tile_pool(name="const", bufs=1))
    ident = const_pool.tile([128, 128], BF16)
    masks.make_identity(nc, ident[:])

    eb_pool = ctx.enter_context(tc.tile_pool(name="eb", bufs=1))
    eb = eb_pool.tile([128, N_TILES, 128], F32)
    nc.sync.dma_start(eb[:], emb.rearrange("(n p) d -> p n d", p=128))
    wo_pool = ctx.enter_context(tc.tile_pool(name="wo", bufs=1))
    wo = wo_pool.tile([128, VS], F32)
    nc.sync.dma_start(wo[:], w_out)

    oh_pool = ctx.enter_context(tc.tile_pool(name="oh", bufs=5))
    sT_pool = ctx.enter_context(tc.tile_pool(name="sT", bufs=4))
    sTps_pool = ctx.enter_context(tc.tile_pool(name="sTps", bufs=3, space="PSUM"))
    acc_pool = ctx.enter_context(tc.tile_pool(name="acc", bufs=2, space="PSUM"))
    avg_pool = ctx.enter_context(tc.tile_pool(name="avg", bufs=2))
    avgg_pool = ctx.enter_context(tc.tile_pool(name="avgg", bufs=3))
    lg_pool = ctx.enter_context(tc.tile_pool(name="lg", bufs=3))
    lgps_pool = ctx.enter_context(tc.tile_pool(name="lgps", bufs=3, space="PSUM"))
    stat_pool = ctx.enter_context(tc.tile_pool(name="stat", bufs=3))
    dram = ctx.enter_context(tc.tile_pool(name="dram", bufs=8, space="DRAM"))
    dram2 = ctx.enter_context(tc.tile_pool(name="dram2", bufs=8, space="DRAM"))

    # per-block state threaded across the pipeline
    st = {}

    def stage1_chunk(bb, j, avgT_ps):
        b0 = bb * BB
        v0 = j * VC
        vc = min(VC, Vs - v0)
        oh = oh_pool.tile([128, Cs, VC], BF16, tag="oh")
        nc.gpsimd.dma_start(oh[:, :, :vc], batch[b0 : b0 + BB, :, v0 : v0 + vc])
        nt = (vc + 127) // 128
        for t in range(nt):
            toff = t * 128
            tw = min(128, vc - toff)
            g = j * (VC // 128) + t
            sT_ps = sTps_pool.tile([128, BB], F32, tag="sTps")
            for c in range(Cs):
                nc.tensor.matmul(
                    sT_ps[:tw],
                    lhsT=oh[:, c, toff : toff + tw],
                    rhs=ident[:],
                    start=(c == 0),
                    stop=(c == Cs - 1),
                )
            sT = sT_pool.tile([128, BB], F32, tag="sT")
            nc.vector.tensor_copy(sT[:tw], sT_ps[:tw])
            nc.tensor.matmul(
                avgT_ps[:],
                lhsT=eb[:tw, g, :],
                rhs=sT[:tw],
                start=(g == 0),
                stop=(g == N_TILES - 1),
            )

    def avg_ar(bb, avgT_ps):
        avgT_sb = avg_pool.tile([128, BB], F32, tag="avg")
        nc.vector.tensor_copy(avgT_sb[:], avgT_ps[:])
        cc_in = dram.tile([128, BB], F32, tag="cc_in")
        cc_out = dram.tile([128, BB], F32, tag="cc_out", addr_space="Shared")
        nc.sync.dma_start(cc_in[:], avgT_sb[:])
        nc.gpsimd.collective_compute(
            "AllReduce",
            mybir.AluOpType.add,
            replica_groups=rg,
            ins=[cc_in.opt()],
            outs=[cc_out.opt()],
        )
        avgT_g = avgg_pool.tile([128, BB], F32, tag="avgg")
        nc.sync.dma_start(avgT_g[:], cc_out[:])
        st[bb] = {"avgT_g": avgT_g}

    def stage2a(bb):
        s = st[bb]
        avgT_g = s["avgT_g"]
        lg = lg_pool.tile([128, VS], BF16, tag="lg")
        sums = stat_pool.tile([128, n_nc], F32, tag="sums")
        for k in range(n_nc):
            n0 = k * NC2
            nw = min(NC2, Vs - n0)
            lg_ps = lgps_pool.tile([128, NC2], F32, tag="lgps")
            nc.tensor.matmul(
                lg_ps[:, :nw],
                lhsT=avgT_g[:],
                rhs=wo[:, n0 : n0 + nw],
                start=True,
                stop=True,
            )
            # exp(0.1 * x): the 1/C mean fold; fused row-sum via accum_out
            nc.scalar.activation(
                lg[:, n0 : n0 + nw],
                lg_ps[:, :nw],
                AF.Exp,
                scale=1.0 / Cs,
                accum_out=sums[:, k : k + 1],
            )
        scr = stat_pool.tile([128, n_nc], F32, tag="scr")
        den = stat_pool.tile([128, 1], F32, tag="den")
        nc.scalar.activation(
            scr[:, :n_nc], sums[:, :n_nc], AF.Copy, accum_out=den[:]
        )
        cc2_in = dram2.tile([128, 1], F32, tag="cc2_in")
        cc2_out = dram2.tile([128, 1], F32, tag="cc2_out", addr_space="Shared")
        nc.sync.dma_start(cc2_in[:], den[:])
        nc.gpsimd.collective_compute(
            "AllReduce",
            mybir.AluOpType.add,
            replica_groups=rg,
            ins=[cc2_in.opt()],
            outs=[cc2_out.opt()],
        )
        s["lg"] = lg
        s["cc2_out"] = cc2_out

    def stage2b(bb):
        s = st.pop(bb)
        lg = s["lg"]
        den_g = stat_pool.tile([128, 1], F32, tag="deng")
        nc.sync.dma_start(den_g[:], s["cc2_out"][:])
        r = stat_pool.tile([128, 1], F32, tag="recip")
        nc.vector.reciprocal(r[:], den_g[:])
        for k in range(n_nc):
            n0 = k * NC2
            nw = min(NC2, Vs - n0)
            nc.scalar.mul(lg[:, n0 : n0 + nw], lg[:, n0 : n0 + nw], r[:])
        b0 = bb * BB
        nc.sync.dma_start(out[b0 : b0 + BB, :], lg[:])

    for bb in range(N_BB):
        avgT_ps = acc_pool.tile([128, BB], F32, tag="acc")
        for j in range(n_vc):
            stage1_chunk(bb, j, avgT_ps)
            if j == 1 and bb >= 1:
                stage2a(bb - 1)
            if j == n_vc - 1 and bb >= 1:
                stage2b(bb - 1)
        avg_ar(bb, avgT_ps)
    stage2a(N_BB - 1)
    stage2b(N_BB - 1)


def build(num_devices=N_CORES):
    nc = bacc.Bacc(
        "TRN2",
        target_bir_lowering=False,
        debug=False,
        num_devices=num_devices,
        num_swdge_queues=4,
    )
    batch = nc.dram_tensor(
        "batch", [B_FULL, C, VS], F32, kind="ExternalInput"
    ).ap()
    emb = nc.dram_tensor("emb", [VS_PAD, D], F32, kind="ExternalInput").ap()
    w_out = nc.dram_tensor("w_out", [D, VS], F32, kind="ExternalInput").ap()
    out = nc.dram_tensor("out", [B_FULL, VS], BF16, kind="ExternalOutput").ap()
    with tile.TileContext(nc) as tc:
        _cbow_kernel(tc, out, batch, emb, w_out)
    nc.compile()
    return nc


_NC = None


def _build_cached():
    global _NC
    if _NC is None:
        _NC = build()
    return _NC


def _run(batch, emb, w_out, trace=False, **kwargs):
    from concourse.bass_utils import run_bass_kernel_spmd

    nc = _build_cached()
    batch = np.ascontiguousarray(np.asarray(batch, dtype=np.float32))
    emb = np.asarray(emb, dtype=np.float32)
    w_out = np.asarray(w_out, dtype=np.float32)
    in_maps = []
    for i in range(N_CORES):
        v0 = i * VS
        emb_pad = np.zeros((VS_PAD, D), dtype=np.float32)
        emb_pad[:VS] = emb[v0 : v0 + VS]
        in_maps.append(
            {
                "batch": np.ascontiguousarray(batch[:, :, v0 : v0 + VS]),
                "emb": emb_pad,
                "w_out": np.ascontiguousarray(w_out[:, v0 : v0 + VS]),
            }
        )
    res = run_bass_kernel_spmd(
        nc, in_maps, core_ids=list(range(N_CORES)), trace=trace, **kwargs
    )
    out = np.concatenate(
        [r["out"].astype(np.float32) for r in res.results], axis=1
    )
    return out, res


def kernel(batch, emb, w_out):
    out, _ = _run(batch, emb, w_out, trace=False)
    return out


# revision 6
# speedup vs baseline: 1.2075x; 1.1980x over previous
"""CBOW (one-hot embedding lookup + mean + output matmul + softmax) on 8
Trainium2 NeuronCores, vocab-sharded end to end.

Full problem: batch [1024, 10, 32000] f32 one-hot, emb [32000, 128] f32,
w_out [128, 32000] f32 -> softmax(mean_c(batch @ emb) @ w_out) [1024, 32000].

Sharding: core i owns vocab columns [i*4000, (i+1)*4000). It receives
  batch_s [1024, 10, 4000] f32  (full batch, its vocab slice)
  emb_s   [4096, 128]      f32  (its emb rows, zero-padded 4000->4096)
  w_out_s [128, 4000]      f32  (its output-projection columns)
and produces out_s [1024, 4000] bf16 (its softmax columns; host concatenates
along vocab and upcasts to f32).

Batch rows run in 8 blocks of 128. Stage 1 streams each block's one-hot
slice through a casting DMA (f32 DRAM -> bf16 SBUF); per 128-wide v-tile
the 10 context planes are summed on the PE as REGULAR bf16 matmuls
(lhsT=oh_c, rhs=identity accumulates oh_c.T in fp32 PSUM) giving sT[v, b],
then sumT_bb[d, b] += emb_tile.T @ sT over the core's 32 v-tiles.

Cross-core reduction is batched into 3 waves of blocks (0-3, 4-5, 6-7) so
only 6 collectives run in total. A collective trigger BLOCKS the issuing
gpsimd queue for the collective's full duration, and gpsimd is also the
only engine that can issue the casting one-hot DMAs - so each trigger is
scheduled (via an explicit event table keyed on (block, chunk)) at a point
where its input is already available and >=3 one-hot chunks are queued
ahead in the DMA rings, letting the SDMA engines keep streaming while
gpsimd waits. Stage 2 (logits matmul; scalar-engine exp(x/C) with fused
row-sum via accum_out; tiny denominator AllReduce per wave; per-partition
scale; bf16 DMA out) is interleaved into later blocks' streaming windows.

Engine roles: gpsimd = one-hot casting DMAs + collective triggers;
sync = producer-side DMAs only (weights, bounce-buffer writes - never
waits on a collective); scalar = collective-output reads, exp/sums/scales,
output writes (all waits naturally in its dependency chain); vector = PSUM
copies + reciprocals only. PE never sees a matmul whose inputs aren't
already resident.
"""

from contextlib import ExitStack

import numpy as np

import concourse.bass as bass
import concourse.tile as tile
from concourse import bacc, masks, mybir
from concourse._compat import with_exitstack

F32 = mybir.dt.float32
BF16 = mybir.dt.bfloat16
AX = mybir.AxisListType
AF = mybir.ActivationFunctionType

B_FULL, C, V, D = 1024, 10, 32000, 128
N_CORES = 8
VS = V // N_CORES          # 4000 vocab columns per core
VS_PAD = 4096              # emb rows padded to a multiple of 128
N_TILES = VS_PAD // 128    # 32 v-tiles (last is 32 valid rows)
BB = 128                   # batch rows per block
N_BB = B_FULL // BB        # 8 blocks
VC = 1024                  # one-hot v-chunk (chunks: 1024,1024,1024,928)
NC2 = 512                  # stage-2 logits chunk

WAVES = [(0, 4), (4, 6), (6, 8)]


@with_exitstack
def _cbow_kernel(ctx: ExitStack, tc, out, batch, emb, w_out):
    nc = tc.nc
    Bs, Cs, Vs = batch.shape
    assert Bs == B_FULL and Cs == C and Vs == VS
    rg = [list(range(N_CORES))]
    n_vc = (Vs + VC - 1) // VC
    n_nc = (Vs + NC2 - 1) // NC2

    const_pool = ctx.enter_context(tc.tile_pool(name="const", bufs=1))
    ident = const_pool.tile([128, 128], BF16)
    masks.make_identity(nc, ident[:])

    eb_pool = ctx.enter_context(tc.tile_pool(name="eb", bufs=1))
    eb = eb_pool.tile([128, N_TILES, 128], F32)
    nc.sync.dma_start(eb[:], emb.rearrange("(n p) d -> p n d", p=128))
    wo_pool = ctx.enter_context(tc.tile_pool(name="wo", bufs=1))
    wo = wo_pool.tile([128, VS], F32)
    nc.sync.dma_start(wo[:], w_out)

    oh_pool = ctx.enter_context(tc.tile_pool(name="oh", bufs=4))
    sT_pool = ctx.enter_context(tc.tile_pool(name="sT", bufs=4))
    sTps_pool = ctx.enter_context(tc.tile_pool(name="sTps", bufs=3, space="PSUM"))
    acc_pool = ctx.enter_context(tc.tile_pool(name="acc", bufs=2, space="PSUM"))
    avgsb_pool = ctx.enter_context(tc.tile_pool(name="avgsb", bufs=2))
    avgg_pool = ctx.enter_context(tc.tile_pool(name="avgg", bufs=2))
    lg_pool = ctx.enter_context(tc.tile_pool(name="lg", bufs=5))
    lgps_pool = ctx.enter_context(tc.tile_pool(name="lgps", bufs=3, space="PSUM"))
    stat_pool = ctx.enter_context(tc.tile_pool(name="stat", bufs=2))
    dram = ctx.enter_context(tc.tile_pool(name="dram", bufs=3, space="DRAM"))

    # wave index -> state dict
    wstate = {}
    for wi, (w0, w1) in enumerate(WAVES):
        cols = (w1 - w0) * BB
        wstate[wi] = {
            "w0": w0,
            "w1": w1,
            "avg_sb": avgsb_pool.tile([128, cols], F32, tag=f"avgsb{wi}", name=f"avgsb{wi}"),
            "den_sb": stat_pool.tile([128, w1 - w0], F32, tag=f"densb{wi}", name=f"densb{wi}"),
            "lg": {},
        }

    bb2wave = {}
    for wi, (w0, w1) in enumerate(WAVES):
        for bb in range(w0, w1):
            bb2wave[bb] = wi

    def stage1_chunk(bb, j, avgT_ps):
        b0 = bb * BB
        v0 = j * VC
        vc = min(VC, Vs - v0)
        oh = oh_pool.tile([128, Cs, VC], BF16, tag="oh")
        nc.gpsimd.dma_start(oh[:, :, :vc], batch[b0 : b0 + BB, :, v0 : v0 + vc])
        nt = (vc + 127) // 128
        for t in range(nt):
            toff = t * 128
            tw = min(128, vc - toff)
            g = j * (VC // 128) + t
            sT_ps = sTps_pool.tile([128, BB], F32, tag="sTps")
            for c in range(Cs):
                nc.tensor.matmul(
                    sT_ps[:tw],
                    lhsT=oh[:, c, toff : toff + tw],
                    rhs=ident[:],
                    start=(c == 0),
                    stop=(c == Cs - 1),
                )
            sT = sT_pool.tile([128, BB], F32, tag="sT")
            nc.vector.tensor_copy(sT[:tw], sT_ps[:tw])
            nc.tensor.matmul(
                avgT_ps[:],
                lhsT=eb[:tw, g, :],
                rhs=sT[:tw],
                start=(g == 0),
                stop=(g == N_TILES - 1),
            )

    def avg_ar(wi):
        """Bounce the wave's context-sums to DRAM, AllReduce, read back."""
        s = wstate[wi]
        cols = (s["w1"] - s["w0"]) * BB
        cc_in = dram.tile([128, cols], F32, tag=f"cc_in{wi}")
        cc_out = dram.tile(
            [128, cols], F32, tag=f"cc_out{wi}", addr_space="Shared"
        )
        nc.sync.dma_start(cc_in[:], s["avg_sb"][:])
        nc.gpsimd.collective_compute(
            "AllReduce",
            mybir.AluOpType.add,
            replica_groups=rg,
            ins=[cc_in.opt()],
            outs=[cc_out.opt()],
        )
        avg_g = avgg_pool.tile([128, cols], F32, tag=f"avgg{wi}")
        nc.scalar.dma_start(avg_g[:], cc_out[:])
        s["avg_g"] = avg_g

    def den_ar(wi):
        s = wstate[wi]
        nb = s["w1"] - s["w0"]
        cc_in = dram.tile([128, nb], F32, tag=f"cc2_in{wi}")
        cc_out = dram.tile(
            [128, nb], F32, tag=f"cc2_out{wi}", addr_space="Shared"
        )
        nc.sync.dma_start(cc_in[:], s["den_sb"][:])
        nc.gpsimd.collective_compute(
            "AllReduce",
            mybir.AluOpType.add,
            replica_groups=rg,
            ins=[cc_in.opt()],
            outs=[cc_out.opt()],
        )
        s["cc2_out"] = cc_out

    def stage2a(bb):
        """Logits + exp (fused row-sum) + this block's local denominator."""
        wi = bb2wave[bb]
        s = wstate[wi]
        slot = bb - s["w0"]
        avg_g = s["avg_g"]
        lg = lg_pool.tile([128, VS], BF16, tag="lg")
        sums = stat_pool.tile([128, n_nc], F32, tag="sums")
        for k in range(n_nc):
            n0 = k * NC2
            nw = min(NC2, Vs - n0)
            lg_ps = lgps_pool.tile([128, NC2], F32, tag="lgps")
            nc.tensor.matmul(
                lg_ps[:, :nw],
                lhsT=avg_g[:, slot * BB : (slot + 1) * BB],
                rhs=wo[:, n0 : n0 + nw],
                start=True,
                stop=True,
            )
            # exp(x / C): the 1/C mean fold; fused row-sum via accum_out
            nc.scalar.activation(
                lg[:, n0 : n0 + nw],
                lg_ps[:, :nw],
                AF.Exp,
                scale=1.0 / Cs,
                accum_out=sums[:, k : k + 1],
            )
        scr = stat_pool.tile([128, n_nc], F32, tag="scr")
        nc.scalar.activation(
            scr[:, :n_nc],
            sums[:, :n_nc],
            AF.Copy,
            accum_out=s["den_sb"][:, slot : slot + 1],
        )
        s["lg"][bb] = lg

    def stage2b(wi):
        """Global denominator -> reciprocal -> scale -> bf16 out."""
        s = wstate[wi]
        nb = s["w1"] - s["w0"]
        den_g = stat_pool.tile([128, nb], F32, tag=f"deng{wi}")
        nc.scalar.dma_start(den_g[:], s["cc2_out"][:])
        r = stat_pool.tile([128, nb], F32, tag=f"recip{wi}")
        nc.vector.reciprocal(r[:], den_g[:])
        for bb in range(s["w0"], s["w1"]):
            slot = bb - s["w0"]
            lg = s["lg"].pop(bb)
            for k in range(n_nc):
                n0 = k * NC2
                nw = min(NC2, Vs - n0)
                nc.scalar.mul(
                    lg[:, n0 : n0 + nw],
                    lg[:, n0 : n0 + nw],
                    r[:, slot : slot + 1],
                )
            b0 = bb * BB
            nc.scalar.dma_start(out[b0 : b0 + BB, :], lg[:])

    # event table: emit these right after stage1_chunk(bb, j)
    events = {
        (4, 3): [lambda: avg_ar(0)],
        (5, 0): [lambda: stage2a(0)],
        (5, 1): [lambda: stage2a(1)],
        (5, 2): [lambda: stage2a(2)],
        (5, 3): [lambda: stage2a(3)],
        (6, 3): [lambda: den_ar(0)],
        (7, 0): [lambda: stage2b(0)],
        (7, 1): [lambda: avg_ar(1)],
        (7, 2): [lambda: stage2a(4)],
        (7, 3): [lambda: stage2a(5)],
    }

    for bb in range(N_BB):
        avgT_ps = acc_pool.tile([128, BB], F32, tag="acc")
        for j in range(n_vc):
            stage1_chunk(bb, j, avgT_ps)
            for fn in events.get((bb, j), []):
                fn()
        # park this block's context-sums in its wave buffer
        wi = bb2wave[bb]
        s = wstate[wi]
        slot = bb - s["w0"]
        nc.vector.tensor_copy(
            s["avg_sb"][:, slot * BB : (slot + 1) * BB], avgT_ps[:]
        )

    # tail: wave 2 reduction + remaining output stages
    avg_ar(2)
    den_ar(1)
    stage2a(6)
    stage2a(7)
    stage2b(1)
    den_ar(2)
    stage2b(2)


def build(num_devices=N_CORES):
    nc = bacc.Bacc(
        "TRN2",
        target_bir_lowering=False,
        debug=False,
        num_devices=num_devices,
        num_swdge_queues=4,
    )
    batch = nc.dram_tensor(
        "batch", [B_FULL, C, VS], F32, kind="ExternalInput"
    ).ap()
    emb = nc.dram_tensor("emb", [VS_PAD, D], F32, kind="ExternalInput").ap()
    w_out = nc.dram_tensor("w_out", [D, VS], F32, kind="ExternalInput").ap()
    out = nc.dram_tensor("out", [B_FULL, VS], BF16, kind="ExternalOutput").ap()
    with tile.TileContext(nc) as tc:
        _cbow_kernel(tc, out, batch, emb, w_out)
    nc.compile()
    return nc


_NC = None


def _build_cached():
    global _NC
    if _NC is None:
        _NC = build()
    return _NC


def _run(batch, emb, w_out, trace=False, **kwargs):
    from concourse.bass_utils import run_bass_kernel_spmd

    nc = _build_cached()
    batch = np.ascontiguousarray(np.asarray(batch, dtype=np.float32))
    emb = np.asarray(emb, dtype=np.float32)
    w_out = np.asarray(w_out, dtype=np.float32)
    in_maps = []
    for i in range(N_CORES):
        v0 = i * VS
        emb_pad = np.zeros((VS_PAD, D), dtype=np.float32)
        emb_pad[:VS] = emb[v0 : v0 + VS]
        in_maps.append(
            {
                "batch": np.ascontiguousarray(batch[:, :, v0 : v0 + VS]),
                "emb": emb_pad,
                "w_out": np.ascontiguousarray(w_out[:, v0 : v0 + VS]),
            }
        )
    res = run_bass_kernel_spmd(
        nc, in_maps, core_ids=list(range(N_CORES)), trace=trace, **kwargs
    )
    out = np.concatenate(
        [r["out"].astype(np.float32) for r in res.results], axis=1
    )
    return out, res


def kernel(batch, emb, w_out):
    out, _ = _run(batch, emb, w_out, trace=False)
    return out
